# revision 21
# baseline (speedup 1.0000x reference)
"""CBAM (channel + spatial attention) Trainium2 kernel, 8-core data parallel.

Problem: f [8, 8, 256, 56, 56] f32 -> out same shape.
  x = f.reshape(BT, C, H, W)
  ca = sigmoid(mlp(max_hw(x)) + mlp(mean_hw(x)));  xc = ca * x
  s  = conv7x7([mean_c(xc); max_c(xc)]);           out = sigmoid(s) * xc

v3 strategy (per NeuronCore, 8 frames each, no collectives):
  - host casts x to bf16 (kernel computes in bf16 anyway) -> HBM traffic
    halves: 1.6MB load + 1.6MB store per frame; out bf16, host upcasts
  - 5-stage software pipeline issued interleaved so each in-order engine
    queue stays close to readiness order
  - stats: pool-max via DVE TT fold trees (fast path; tensor_reduce and
    scalar_tensor_tensor are ~2x slow paths on this HW); pool-avg via ACT
    copy+accum_out into a write-only scratch (ACT is cheap capacity)
  - xc = ca*x via DVE tensor_scalar (fastest bulk op, ~0.33ns/elem)
  - spatial sum: 14 accumulating PE matmuls with per-frame ca-masked lhsT
    into one PSUM bank [7,448]; one ACT copy; one DMA into the padded
    conv input
  - spatial max: m1=max(xc0,xc1) written 112-aligned into [128,28,128];
    one DMA XBAR transpose -> [128(pix),28,128(chan)]; DVE TT fold tree
    over channels -> [128,28]; tiny PE transpose + ACT copy -> s_pad
  - conv 7x7 as 7 accumulating PE matmuls with host-built banded lhsT
  - sa broadcast via PE ones-matmul + ACT PSUM copies, hoisted one stage
    so the copies run at iteration start instead of serializing the tail
  - loads+XBAR on sync HWDGE queue, stores on gpsimd SWDGE queue
"""

import sys
from contextlib import ExitStack

import numpy as np

if "/opt/trn_rl_repo" not in sys.path:
    sys.path.insert(0, "/opt/trn_rl_repo")

import ml_dtypes

import concourse.bass as bass
import concourse.tile as tile
from concourse import bacc, mybir
from concourse.bass_utils import run_bass_kernel_spmd
from concourse.masks import make_identity

F32 = mybir.dt.float32
BF16 = mybir.dt.bfloat16
BFNP = ml_dtypes.bfloat16

N_CORES = 8
B, T, C, H, W = 8, 8, 256, 56, 56
HW = H * W            # 3136
FRAMES = B * T        # 64
FPC = FRAMES // N_CORES  # frames per core = 8
PAD = 3
HP, WP = H + 2 * PAD, W + 2 * PAD  # 62, 62
CHK = 112             # real pixels per xbar chunk (28 * 112 = 3136)
NCHUNK = HW // CHK    # 28
XW = 128              # xbar chunk stride (pad 112 -> 128)
SCHK = 448            # free-dim chunk for ssum matmuls (one PSUM bank row)
NSCHK = HW // SCHK    # 7


def _build_conv_lhsT(conv_w: np.ndarray) -> np.ndarray:
    """Banded matrices for the 7x7 conv as 7 accumulating matmuls over y.

    cb[p, dx, yo] with p = c*62 + y_in: w_eff[c, yi-yo, dx] for 0<=yi-yo<=6.
    The channel-mean 1/C is folded into the avg branch (c=0). bf16 output.
    """
    w_eff = conv_w[0].astype(np.float64).copy()  # [2, 7, 7]
    w_eff[0] /= C
    Bm = np.zeros((7, 2 * HP, H), dtype=np.float32)
    dyi = np.arange(7)
    for dx in range(7):
        for c in range(2):
            for yo in range(H):
                Bm[dx, c * HP + yo + dyi, yo] = w_eff[c, :, dx]
    return np.ascontiguousarray(Bm.transpose(1, 0, 2)).astype(BFNP)  # [124,7,56]


def build_nc(n_frames: int = FPC):
    nc = bacc.Bacc("TRN2", target_bir_lowering=False, debug=False,
                   num_devices=N_CORES)

    x_ext = nc.dram_tensor("x", [n_frames, C, HW], BF16, kind="ExternalInput")
    w1_ext = nc.dram_tensor("w1", [C, 16], F32, kind="ExternalInput")
    w2_ext = nc.dram_tensor("w2", [16, C], F32, kind="ExternalInput")
    cb_ext = nc.dram_tensor("convb", [2 * HP, 7, H], BF16, kind="ExternalInput")
    out_ext = nc.dram_tensor("out", [n_frames, C, HW], BF16,
                             kind="ExternalOutput")
    # scratch for the sa broadcast: sa goes to DRAM, then a stride-0 read
    # replicates it across 128 partitions (dispatched a full iteration after
    # the write, so the untracked DRAM dependency cannot race)
    sa_dram = nc.dram_tensor("sad", [n_frames, HW], BF16, kind="Internal")

    AF = mybir.ActivationFunctionType
    ALU = mybir.AluOpType
    AX = mybir.AxisListType

    with tile.TileContext(nc) as tc, ExitStack() as ctx:
        consts = ctx.enter_context(tc.tile_pool(name="consts", bufs=1))
        xin = ctx.enter_context(tc.tile_pool(name="xin", bufs=3))
        scrp = ctx.enter_context(tc.tile_pool(name="scr", bufs=1))
        xcp = ctx.enter_context(tc.tile_pool(name="xc", bufs=3))
        m1p = ctx.enter_context(tc.tile_pool(name="m1", bufs=3))
        trp = ctx.enter_context(tc.tile_pool(name="tr", bufs=3))
        sabp = ctx.enter_context(tc.tile_pool(name="sab", bufs=2))
        obp = ctx.enter_context(tc.tile_pool(name="ob", bufs=2))
        small = ctx.enter_context(tc.tile_pool(name="small", bufs=2))
        cap = ctx.enter_context(tc.tile_pool(name="cap", bufs=3))
        rowp = ctx.enter_context(tc.tile_pool(name="rowp", bufs=3))
        spadp = ctx.enter_context(tc.tile_pool(name="spad", bufs=2))
        fold = ctx.enter_context(tc.tile_pool(name="fold", bufs=1))
        # PSUM pools — bank budget: pss 2 + pz 2 + pmlp 2 = 6
        pss = ctx.enter_context(tc.tile_pool(name="pss", bufs=2, space="PSUM"))
        pz = ctx.enter_context(tc.tile_pool(name="pz", bufs=2, space="PSUM"))
        pmlp = ctx.enter_context(tc.tile_pool(name="pmlp", bufs=2, space="PSUM"))

        # ---- constants / weights (loaded once) ----
        w1_sb = consts.tile([128, 2, 16], F32)       # [k, ktile, m]
        nc.sync.dma_start(w1_sb[:], w1_ext.rearrange("(t p) m -> p t m", t=2))
        w2_sb = consts.tile([16, C], F32)
        nc.sync.dma_start(w2_sb[:], w2_ext[:, :])
        cb_sb = consts.tile([124, 7, H], BF16)       # [y_in(+c), dx, y_out]
        nc.sync.dma_start(cb_sb[:], cb_ext[:, :, :])
        ident_b = consts.tile([128, 128], BF16)
        make_identity(nc, ident_b[:])
        # mask7[:, j, :]: [128, 7] lhsT with ones in column j only; scaled by
        # ca each frame so 14 accumulating matmuls land each hw-chunk's
        # weighted channel-sum in its own row of one PSUM bank.
        mask7 = consts.tile([128, NSCHK, NSCHK], BF16)
        nc.vector.memset(mask7[:], 0.0)
        for j in range(NSCHK):
            nc.vector.memset(mask7[:, j, j:j + 1], 1.0)

        state = [dict() for _ in range(n_frames)]

        def s_load(f):
            st = state[f]
            x_sb = xin.tile([128, 2, HW], BF16, tag="x")
            nc.sync.dma_start(x_sb[:], x_ext[f].rearrange("(t p) w -> p t w",
                                                          t=2))
            st["x"] = x_sb

        def s2a_xc(f):
            """xc = ca*x: both groups on ACT (copy with per-partition scale)."""
            st = state[f]
            x_sb, ca = st["x"], st["ca"]
            xc = xcp.tile([128, 2, HW], BF16, tag="xc")
            m1 = m1p.tile([128, NCHUNK, XW], BF16, tag="m1")
            nc.gpsimd.memset(m1[:, :, CHK:XW], 0.0)
            for t in range(2):
                nc.scalar.activation(xc[:, t, :], x_sb[:, t, :], AF.Copy,
                                     scale=ca[:, t:t + 1])
            st["xc"] = xc
            st["m1"] = m1

        def s2a_m1(f):
            """m1 = max(xc0, xc1), written 112-aligned for the XBAR."""
            st = state[f]
            xc, m1 = st["xc"], st["m1"]
            for hh in range(2):
                cs = slice(hh * (NCHUNK // 2), (hh + 1) * (NCHUNK // 2))
                sl = slice(hh * (HW // 2), (hh + 1) * (HW // 2))
                nc.vector.tensor_tensor(
                    out=m1[:, cs, 0:CHK],
                    in0=xc[:, 0, sl].rearrange("p (c w) -> p c w", w=CHK),
                    in1=xc[:, 1, sl].rearrange("p (c w) -> p c w", w=CHK),
                    op=ALU.max)

        def s2a_ssum(f):
            """spatial-sum matmuls + PSUM copy (inputs one iteration old)."""
            st = state[f]
            psc = pss.tile([NSCHK, SCHK], F32, tag="pss")
            nmm = 2 * NSCHK
            k = 0
            for t in range(2):
                for j in range(NSCHK):
                    nc.tensor.matmul(psc[:], st["camask"][:, t, j, :],
                                     st["x"][:, t, j * SCHK:(j + 1) * SCHK],
                                     start=(k == 0), stop=(k == nmm - 1))
                    k += 1
            ssum_sb = rowp.tile([NSCHK, 8, H], BF16, tag="ssum_sb")
            nc.scalar.activation(ssum_sb[:], psc[:], AF.Copy)
            st["ssum_sb"] = ssum_sb

        def s2a_xbar(f):
            """XBAR transpose dispatch — last on the sync queue so its wait
            on m1 never blocks loads or the small conv-assembly DMAs."""
            st = state[f]
            tr = trp.tile([128, NCHUNK, XW], BF16, tag="tr")
            nc.sync.dma_start_transpose(
                tr[:], st["m1"][:].rearrange("p c w -> p (c w)"))
            st["tr"] = tr

        def s1_stats(f):
            """pool max (DVE fold trees) + pool avg (ACT copy+accum)."""
            st = state[f]
            pr = small.tile([128, 4], F32, tag="pr")
            for t in range(2):
                l1 = fold.tile([128, HW // 2], BF16, tag=f"l1_{t}")
                nc.vector.tensor_tensor(
                    out=l1[:], in0=st["x"][:, t, 0:HW // 2],
                    in1=st["x"][:, t, HW // 2:HW], op=ALU.max)
                l2 = fold.tile([128, HW // 4], BF16, tag=f"l2_{t}")
                nc.vector.tensor_tensor(
                    out=l2[:], in0=l1[:, 0:HW // 4], in1=l1[:, HW // 4:HW // 2],
                    op=ALU.max)
                nc.vector.tensor_reduce(
                    out=pr[:, 2 * t:2 * t + 1], in_=l2[:], axis=AX.X,
                    op=ALU.max)
            scr = scrp.tile([128, 2, HW], BF16, tag="scr")
            for t in range(2):
                nc.scalar.activation(
                    scr[:, t, :], st["x"][:, t, :], AF.Copy,
                    accum_out=pr[:, 2 * t + 1:2 * t + 2])
            st["pr"] = pr

        def s3_bcast(f):
            """sa broadcast: stride-0 DMA read replicating the DRAM row."""
            st = state[f]
            sab = sabp.tile([128, HW], BF16, tag="sab")
            nc.sync.dma_start(
                sab[:], sa_dram[f].unsqueeze(0).to_broadcast((128, HW)))
            st["sab"] = sab

        def s3_final(f):
            """ob = xc * sab, store via gpsimd SWDGE queue."""
            st = state[f]
            ob = obp.tile([128, 2, HW], BF16, tag="ob")
            for t in range(2):
                nc.vector.tensor_tensor(out=ob[:, t, :], in0=st["xc"][:, t, :],
                                        in1=st["sab"][:], op=ALU.mult)
                nc.gpsimd.dma_start(out_ext[f, t * 128:(t + 1) * 128, :],
                                    ob[:, t, :])

        def s2b_spatial(f):
            """spatial max fold + conv input assembly + conv + sa (stage k-3:
            all inputs (tr, ssum_sb) are one iteration old — no queue stalls)."""
            st = state[f]
            s_pad = spadp.tile([124, WP], BF16, tag="s_pad")
            nc.gpsimd.memset(s_pad[:], 0.0)

            # spatial max: fold tree over channels in the transposed layout
            tr = st["tr"]
            w = XW
            cur = tr[:, :, :]
            while w > 8:
                nxt = fold.tile([128, NCHUNK, w // 2], BF16, tag=f"sf{w}")
                nc.vector.tensor_tensor(out=nxt[:], in0=cur[:, :, 0:w // 2],
                                        in1=cur[:, :, w // 2:w], op=ALU.max)
                cur = nxt[:, :, :]
                w //= 2
            rs = small.tile([128, NCHUNK], BF16, tag="rs")
            nc.vector.tensor_reduce(out=rs[:], in_=cur, axis=AX.X, op=ALU.max)
            psm = pz.tile([NCHUNK, 128], BF16, tag="pz")
            nc.tensor.transpose(psm[:], rs[:], ident_b[:])
            sm_sb = small.tile([NCHUNK, 2, H], BF16, tag="sm_sb")
            nc.scalar.activation(sm_sb[:], psm[:, 0:CHK], AF.Copy)

            # conv input assembly (both DMAs ahead of the XBAR on sync queue)
            nc.sync.dma_start(s_pad[PAD:PAD + H, PAD:PAD + W], st["ssum_sb"][:])
            nc.sync.dma_start(s_pad[HP + PAD:HP + PAD + H, PAD:PAD + W],
                              sm_sb[:])

            # conv: 7 accumulating matmuls
            pzt = pz.tile([H, W], F32, tag="pz")
            for dx in range(7):
                nc.tensor.matmul(pzt[:], cb_sb[:, dx, :],
                                 s_pad[:, dx:dx + W],
                                 start=(dx == 0), stop=(dx == 6))
            sa_yx = small.tile([H, W], BF16, tag="sa_yx")
            nc.scalar.activation(sa_yx[:], pzt[:], AF.Sigmoid)
            nc.sync.dma_start(sa_dram[f, :], sa_yx[:])

        def s1_mlp(f):
            """shared MLP -> ca + per-frame ca-masked ssum lhsT."""
            st = state[f]
            pr = st["pr"]
            ph = pmlp.tile([16, 2], F32, tag="pmlp")
            for t in range(2):
                nc.tensor.matmul(ph[:], w1_sb[:, t, :],
                                 pr[:, 2 * t:2 * t + 2],
                                 start=(t == 0), stop=(t == 1))
            h = small.tile([16, 2], F32, tag="h")
            # avg branch carries raw sums; undo with 1/HW before relu
            nc.scalar.activation(h[:, 0:1], ph[:, 0:1], AF.Relu)
            nc.scalar.activation(h[:, 1:2], ph[:, 1:2], AF.Relu,
                                 scale=1.0 / HW)
            hs = small.tile([16, 1], F32, tag="hs")
            nc.vector.tensor_tensor(out=hs[:], in0=h[:, 0:1], in1=h[:, 1:2],
                                    op=ALU.add)
            pca = pmlp.tile([128, 2], F32, tag="pmlp")
            for t in range(2):
                nc.tensor.matmul(pca[:, t:t + 1],
                                 w2_sb[:, t * 128:(t + 1) * 128], hs[:],
                                 start=True, stop=True)
            ca = cap.tile([128, 2], F32, tag="ca")
            nc.scalar.activation(ca[:], pca[:], AF.Sigmoid)
            camask = cap.tile([128, 2, NSCHK, NSCHK], BF16, tag="camask")
            for t in range(2):
                nc.vector.tensor_scalar(
                    out=camask[:, t, :, :], in0=mask7[:],
                    scalar1=ca[:, t:t + 1], scalar2=None, op0=ALU.mult)
            st["ca"] = ca
            st["camask"] = camask

        # 6-deep software pipeline:
        # load(k) | stats/mlp(k-1) | xc/m1/ssum/XBAR(k-2) | spatial(k-3)
        #         | bcast+final(k-4)
        for k in range(n_frames + 4):
            if k < n_frames:
                s_load(k)
            if 4 <= k:
                s3_bcast(k - 4)
            if 2 <= k <= n_frames + 1:
                s2a_xc(k - 2)
                s2a_ssum(k - 2)
            if 1 <= k <= n_frames:
                s1_stats(k - 1)
            if 3 <= k <= n_frames + 2:
                s2b_spatial(k - 3)
            if 2 <= k <= n_frames + 1:
                s2a_m1(k - 2)
            if 4 <= k:
                s3_final(k - 4)
            if 2 <= k <= n_frames + 1:
                s2a_xbar(k - 2)
            if 1 <= k <= n_frames:
                s1_mlp(k - 1)

    nc.finalize()  # bacc register allocation + DCE (bass2jax expects this)
    return nc


_NC_CACHE = {}


def _get_nc(n_frames: int):
    if n_frames not in _NC_CACHE:
        _NC_CACHE[n_frames] = build_nc(n_frames)
    return _NC_CACHE[n_frames]


def kernel(f: np.ndarray, w1: np.ndarray, w2: np.ndarray,
           conv_w: np.ndarray) -> np.ndarray:
    f = np.asarray(f, dtype=np.float32)
    w1 = np.ascontiguousarray(np.asarray(w1, dtype=np.float32))
    w2 = np.ascontiguousarray(np.asarray(w2, dtype=np.float32))
    conv_w = np.asarray(conv_w, dtype=np.float32)

    convb = _build_conv_lhsT(conv_w)
    frames = f.reshape(FRAMES, C, HW).astype(BFNP)

    nc = _get_nc(FPC)
    in_maps = []
    for i in range(N_CORES):
        in_maps.append({
            "x": np.ascontiguousarray(frames[i * FPC:(i + 1) * FPC]),
            "w1": w1,
            "w2": w2,
            "convb": convb,
        })
    res = run_bass_kernel_spmd(nc, in_maps, core_ids=list(range(N_CORES)))
    out = np.concatenate([res.results[i]["out"] for i in range(N_CORES)],
                         axis=0)
    return out.reshape(B, T, C, H, W).astype(np.float32)


if __name__ == "__main__":
    rng = np.random.default_rng(0)
    f = rng.standard_normal((B, T, C, H, W), dtype=np.float32)
    w1 = rng.standard_normal((C, 16), dtype=np.float32) / 16.0
    w2 = rng.standard_normal((16, C), dtype=np.float32) / 4.0
    conv_w = rng.standard_normal((1, 2, 7, 7), dtype=np.float32) * 0.1
    out = kernel(f, w1, w2, conv_w)
    print("kernel ran, out shape", out.shape, out.dtype)
